# revision 14
# baseline (speedup 1.0000x reference)
"""Trainium2 Bass kernel for MyMultiAttentionLayer.

Model (reference):
    q = einsum('bsd,hpd->bhsp', x, q_w) + q_b      (same for k, v)
    scores = q @ k^T / sqrt(P)                      [B,H,S,S]
    attn = softmax(scores, axis=2)                  # softmax over the QUERY axis
    ctx = einsum('bhqk,bhkp->bqhp', attn, v)
    out = concat(ctx) @ l_w.T + l_b                 [B,S,NUM_OUT]

Shapes: B=2, S=2048, D=1024, H=16, P=64, NUM_OUT=1024.

Sharding: 8 cores = 2 batches x 4 head-groups (4 heads each).  Each core
computes its batch's attention for its 4 heads plus the partial output
projection over its 256 features; the host sums the partials per batch
(all-reduce equivalent) and adds l_b.

Softmax is over the query axis, so the normalizer Z[k] = sum_q exp(s[q,k])
depends only on k: ctx = sum_k e[q,k]*(v[k,:]/Z[k]) — the normalization is
folded into the 64-wide v rows instead of the 2048-wide attention matrix.
The q bias drops out entirely (its score terms are constant along the
softmax axis); only the k bias is kept.

Design (v2) — the Scalar engine (exp) is the pacer at ~2.5us per ki-tile
(2x activation [128,1024] + accumulator reads); everything else is
scheduled to hide underneath it:
  * ctx accumulates in PSUM across all 16 ki-tiles of a head (4 chunk
    slices in the partition halves of one [128,1024] psum tile), so the
    per-group vector adds of v1 disappear; one drain per head.
  * pair-0 q AND k projections both run in the lead d-loop (all 8 psum
    banks), DMA-paced; q is drained by the otherwise-idle Scalar engine,
    k by Vector (with bias).
  * fillers keep the PE warm (HAM halves the PE clock when it idles):
    v-projection in head 0, pair-1 q/k in head 1, the pair-0 half of the
    output projection (K=128: both heads' ctx stacked) in heads 2/3,
    dummy matmuls for any remaining slack.
  * output = two bf16 partial tensors (pair0 streamed during heads 2/3,
    pair1 in the tail with drains split Scalar/Vector); host sums 8
    partials per batch and adds l_b.
"""

import numpy as np

import concourse.bass as bass
import concourse.tile as tile
from concourse import bacc, mybir
from concourse.bass_utils import run_bass_kernel_spmd

B, S, D = 2, 2048, 1024
H, P = 16, 64
NUM_OUT = 1024
N_CORES = 8
HPC = 4                 # heads per core
PAIRS = 2               # head pairs per core (2 heads x 64 = 128 partitions)
DT = D // 128           # 8 d-tiles
ST = S // 128           # 16 s-tiles
SC = S // 512           # 4 s-chunks of 512
NC_CH = NUM_OUT // 512  # 2 output chunks

F32 = mybir.dt.float32
F16 = mybir.dt.float16
BF16 = mybir.dt.bfloat16
EXP = mybir.ActivationFunctionType.Exp
COPY = mybir.ActivationFunctionType.Copy


def build_nc():
    nc = bacc.Bacc("TRN2", target_bir_lowering=False, debug=False,
                   num_devices=N_CORES)

    xt_d = nc.dram_tensor("xt", [D, S], F16, kind="ExternalInput")
    qwT_d = nc.dram_tensor("qwT", [D, HPC * P], F16, kind="ExternalInput")
    kwT_d = nc.dram_tensor("kwT", [D, HPC * P], F16, kind="ExternalInput")
    vwT_d = nc.dram_tensor("vwT", [D, HPC * P], F16, kind="ExternalInput")
    kb_d = nc.dram_tensor("kb", [HPC * P, 1], F32, kind="ExternalInput")
    vb_d = nc.dram_tensor("vb", [1, HPC * P], F16, kind="ExternalInput")
    lwT_d = nc.dram_tensor("lwT", [HPC * P, NUM_OUT], F16, kind="ExternalInput")
    ones_d = nc.dram_tensor("ones", [1, 128], F16, kind="ExternalInput")
    out0_d = nc.dram_tensor("out0", [S, NUM_OUT], BF16, kind="ExternalOutput")
    out1_d = nc.dram_tensor("out1", [S, NUM_OUT], BF16, kind="ExternalOutput")

    with tile.TileContext(nc) as tc:
        with (
            tc.tile_pool(name="qk", bufs=4) as p_qk,
            tc.tile_pool(name="vv", bufs=ST) as p_v,
            tc.tile_pool(name="cst", bufs=1) as p_c,
            tc.tile_pool(name="zz", bufs=6) as p_z,
            tc.tile_pool(name="et", bufs=4) as p_et,
            tc.tile_pool(name="cc", bufs=2) as p_cc,
            tc.tile_pool(name="ob", bufs=3) as p_ob,
            tc.tile_pool(name="xt", bufs=DT) as p_xt,
            tc.tile_pool(name="wst", bufs=3 * DT) as p_w,
            tc.tile_pool(name="mm", bufs=2, space=bass.MemorySpace.PSUM) as p_mm,
            tc.tile_pool(name="cx", bufs=1, space=bass.MemorySpace.PSUM) as p_cx,
            tc.tile_pool(name="pf", bufs=2, space=bass.MemorySpace.PSUM) as p_pf,
        ):
            # ---- stage inputs, in the order the PE needs them ----
            # Small constants first, then the d-loop stream (qw/kw/xt per
            # d-tile).  vw and lw DMAs are gated behind late xt tiles (via a
            # WAW dependency on the destination tile) so the DMA rings spend
            # the early bandwidth on the critical path.
            kb_t = []
            for pr in range(PAIRS):
                t = p_c.tile([128, 1], F32, name=f"kb{pr}", tag=f"kb{pr}")
                nc.sync.dma_start(t[:], kb_d[pr * 128:(pr + 1) * 128, :])
                kb_t.append(t)
            vb_t = p_c.tile([1, HPC * P], F16, name="vb", tag="vb")
            nc.sync.dma_start(vb_t[:], vb_d[:, :])
            ones = p_c.tile([1, 128], F16, name="ones", tag="ones")
            nc.sync.dma_start(ones[:], ones_d[:, :])
            # xt tiles split into 4 column-slice DMAs: one dma_start maps to
            # one DMA ring, so splitting the big critical transfers across
            # rings is what actually buys bandwidth, and the per-ring FIFO
            # keeps completion roughly in issue order.
            xt, wq, wk, wv = [], [], [], []
            for d in range(DT):
                t = p_w.tile([128, HPC * P], F16, name=f"qw{d}", tag="w")
                nc.sync.dma_start(t[:], qwT_d[d * 128:(d + 1) * 128, :])
                wq.append(t)
                t = p_w.tile([128, HPC * P], F16, name=f"kw{d}", tag="w")
                nc.sync.dma_start(t[:], kwT_d[d * 128:(d + 1) * 128, :])
                wk.append(t)
                t = p_xt.tile([128, S], F16, name=f"xt{d}", tag="xt")
                for c in range(SC):
                    nc.sync.dma_start(
                        t[:, c * 512:(c + 1) * 512],
                        xt_d[d * 128:(d + 1) * 128, c * 512:(c + 1) * 512])
                xt.append(t)
            for d in range(DT):
                t = p_w.tile([128, HPC * P], F16, name=f"vw{d}", tag="w")
                nc.vector.tensor_copy(t[0:1, 0:1], xt[5][0:1, 0:1])
                nc.sync.dma_start(t[:], vwT_d[d * 128:(d + 1) * 128, :])
                wv.append(t)
            lw_t = []
            for pr in range(PAIRS):
                t = p_c.tile([128, NUM_OUT], F16, name=f"lw{pr}", tag=f"lw{pr}")
                nc.vector.tensor_copy(t[0:1, 0:1], xt[7][0:1, 0:1])
                nc.sync.dma_start(t[:], lwT_d[pr * 128:(pr + 1) * 128, :])
                lw_t.append(t)

            # SBUF destinations for the projections
            qkT = {"q": [], "k": []}
            for nm in ("q", "k"):
                for pr in range(PAIRS):
                    qkT[nm].append(p_qk.tile([128, S], F16,
                                             name=f"{nm}T{pr}", tag="qk"))
            v_t = [p_v.tile([128, HPC * P], F16, name=f"v{st}", tag="v")
                   for st in range(ST)]
            # paired ctx accumulators [128=(2 heads x 64p), S] fp16
            acc_t = [p_cc.tile([128, S], F16, name=f"acc{pr}", tag="cc")
                     for pr in range(PAIRS)]

            # ---- pair-0 q AND k projection: d-outer over all 8 chunks so
            # every matmul issues as soon as its xt d-tile DMA lands.
            # q gets the two mm tiles, k gets cx + the two pf slots.
            ps_q = [p_mm.tile([128, 1024], F32, name=f"pp0_q{i}", tag="mm")
                    for i in range(2)]
            ps_kc = p_cx.tile([128, 1024], F32, name="pp0_k01", tag="cx")
            ps_kp = [p_pf.tile([128, 512], F32, name=f"pp0_k{2 + i}", tag="pf")
                     for i in range(2)]
            k_dst = [ps_kc[:, 0:512], ps_kc[:, 512:1024],
                     ps_kp[0][:], ps_kp[1][:]]
            for d in range(DT):
                lq = wq[d][:, 0:128]
                lk = wk[d][:, 0:128]
                for c in range(SC):
                    nc.tensor.matmul(
                        ps_q[c // 2][:, (c % 2) * 512:(c % 2) * 512 + 512],
                        lq, xt[d][:, c * 512:(c + 1) * 512],
                        start=(d == 0), stop=(d == DT - 1))
                    nc.tensor.matmul(
                        k_dst[c], lk, xt[d][:, c * 512:(c + 1) * 512],
                        start=(d == 0), stop=(d == DT - 1))
            # drains: k chunk 0 first (vector, +bias); q on the idle Scalar
            # engine; k chunks 1-3 on vector.
            nc.vector.tensor_scalar_add(
                qkT["k"][0][:, 0:512], ps_kc[:, 0:512], kb_t[0][:])
            for i in range(2):
                nc.scalar.activation(
                    qkT["q"][0][:, i * 1024:(i + 1) * 1024], ps_q[i][:], COPY)
            nc.vector.tensor_scalar_add(
                qkT["k"][0][:, 512:1024], ps_kc[:, 512:1024], kb_t[0][:])
            for c in (2, 3):
                nc.vector.tensor_scalar_add(
                    qkT["k"][0][:, c * 512:(c + 1) * 512], ps_kp[c - 2][:],
                    kb_t[0][:])
            del ps_q, ps_kc, ps_kp, k_dst

            # ---- projection work units (transient PSUM, usable as filler)
            def qk_unit(nm, pr, c):
                wts = wq if nm == "q" else wk
                ps = p_pf.tile([128, 512], F32, name=f"pp_{nm}{pr}{c}",
                               tag="pf")
                for d in range(DT):
                    nc.tensor.matmul(
                        ps[:], wts[d][:, pr * 128:(pr + 1) * 128],
                        xt[d][:, c * 512:(c + 1) * 512],
                        start=(d == 0), stop=(d == DT - 1))
                if nm == "q":
                    nc.vector.tensor_copy(
                        qkT["q"][pr][:, c * 512:(c + 1) * 512], ps[:])
                else:
                    nc.vector.tensor_scalar_add(
                        qkT["k"][pr][:, c * 512:(c + 1) * 512], ps[:],
                        kb_t[pr][:])

            def v_unit(st):
                # v[s, hp] = sum_d xt[d, s] * vwT[d, hp]; bias added in the
                # drain (vb_full is vb broadcast to all 128 partitions).
                ps = p_pf.tile([128, 512], F32, name=f"pp_v{st}", tag="pf")
                for d in range(DT):
                    nc.tensor.matmul(
                        ps[:, :HPC * P],
                        xt[d][:, st * 128:(st + 1) * 128], wv[d][:],
                        start=(d == 0), stop=(d == DT - 1))
                nc.vector.tensor_add(v_t[st][:], ps[:, :HPC * P], vb_full[:])

            # pair-0 output projection partial (K=128: both heads' ctx
            # stacked), computed as PE filler during heads 2/3.
            def out_unit(pr, st, out_d_):
                ob = p_ob.tile([128, NUM_OUT], BF16, name=f"ob{pr}_{st}",
                               tag="ob")
                for ncn in range(NC_CH):
                    po = p_pf.tile([128, 512], F32, name=f"po{pr}_{st}{ncn}",
                                   tag="pf")
                    nc.tensor.matmul(
                        po[:], acc_t[pr][:, st * 128:(st + 1) * 128],
                        lw_t[pr][:, ncn * 512:(ncn + 1) * 512],
                        start=True, stop=True)
                    if pr == 0:
                        nc.vector.tensor_copy(
                            ob[:, ncn * 512:(ncn + 1) * 512], po[:])
                    else:
                        # tail: split drains between Scalar and Vector
                        if st % 2 == 0:
                            nc.scalar.activation(
                                ob[:, ncn * 512:(ncn + 1) * 512], po[:], COPY)
                        else:
                            nc.vector.tensor_copy(
                                ob[:, ncn * 512:(ncn + 1) * 512], po[:])
                nc.sync.dma_start(out_d_[st * 128:(st + 1) * 128, :], ob[:])

            ndum = [0]

            def dummy_unit(n=2):
                # keep-warm matmuls: the HAM activity monitor halves the PE
                # clock whenever the PE idles.
                ps = p_pf.tile([128, 512], F32, name=f"pp_d{ndum[0]}",
                               tag="pf")
                ndum[0] += 1
                for i in range(n):
                    nc.tensor.matmul(ps[:], xt[0][:, :128], xt[0][:, :512],
                                     start=(i == 0), stop=(i == n - 1))

            def vb_unit():
                # vb broadcast to all partitions (ones^T @ vb), used by the
                # v-projection drains
                ps_vb = p_pf.tile([128, 512], F32, name="ps_vb", tag="pf")
                nc.tensor.matmul(ps_vb[:, :HPC * P], ones[:], vb_t[:],
                                 start=True, stop=True)
                nc.vector.tensor_copy(vb_full[:], ps_vb[:, :HPC * P])

            vb_full = p_c.tile([128, HPC * P], F16, name="vbf", tag="vbf")

            # filler schedules per head: each entry is a list of callables
            # consumed one per ki-tile iteration.  Iteration 0 of heads 1-3
            # is left empty: the previous head's ctx flush + drain lands
            # there.  Dummy units keep the PE warm in the slack (the HAM
            # activity monitor halves the PE clock when the PE idles, which
            # would double the cost of every later matmul).
            vu = lambda st: (lambda: v_unit(st))
            ou = lambda st: (lambda st=st: out_unit(0, st, out0_d))
            qu = lambda nm, c: (lambda: qk_unit(nm, 1, c))
            du = lambda n: (lambda: dummy_unit(n))

            def multi(*us):
                def f():
                    for u in us:
                        u()
                return f

            # head 0: v projection, just in time (vs of tile t needs v_t[t];
            # iterations 0/1 have no ctx matmuls yet, so they take doubles).
            fill_h = {}
            fill_h[0] = [multi(vb_unit, vu(0), vu(1)), multi(vu(2), vu(3))] + \
                        [vu(st) for st in range(4, ST)] + [du(2), du(2)]
            # head 1: pair-1 q (needed at head-2 start), then pair-1 k
            # chunks 0-1 (kT1 tile t is needed at head-2 iteration t; chunk
            # c covers tiles 4c..4c+3).  k chunks 2-3 slide to head 2.
            fill_h[1] = [None, qu("q", 0), du(3), qu("q", 1), du(3),
                         qu("q", 2), du(3), qu("q", 3), du(3), qu("k", 0),
                         du(3), qu("k", 1), du(3), du(3), du(3), du(3)]
            # heads 2/3: pair-0 output projection (+ k1 chunks 2-3 early in
            # head 2).
            fill_h[2] = [None, qu("k", 2), ou(0), qu("k", 3), ou(1), ou(2),
                         du(3), ou(3), du(3), ou(4), du(3), ou(5), du(3),
                         ou(6), du(3), ou(7)]
            fill_h[3] = [None, ou(8), du(4), ou(9), du(4), ou(10), du(4),
                         ou(11), du(4), ou(12), du(4), ou(13), du(4),
                         ou(14), ou(15), du(4)]

            # ---- attention: per ki-tile: 2 score matmuls -> 2 wide exps
            # (Z fused) -> 4 ctx matmuls (lag 2) accumulating in PSUM over
            # all 16 ki-tiles.  The previous head's last ctx group + drain
            # are flushed after the next head's first scores so the PE never
            # waits on the vector drain.
            flush_prev = [None]
            for h in range(HPC):
                pr, off = divmod(h, 2)
                off *= 64
                kT, qT = qkT["k"][pr], qkT["q"][pr]
                fill = fill_h[h]
                cxa = p_cx.tile([128, 1024], F32, name=f"cx{h}", tag="cx")
                hist = []  # (ets, vs) per ki-tile, ctx emitted with lag 2

                def emit_ctx(t, cxa=cxa, hist=hist):
                    # cxa partitions 0:64 = q 0..1023, 64:128 = q 1024..2047
                    ets_, vs_ = hist[t]
                    for c in range(SC):
                        dst = cxa[(c // 2) * 64:(c // 2) * 64 + 64, :]
                        nc.tensor.matmul(
                            dst[:, (c % 2) * 512:(c % 2) * 512 + 512],
                            vs_[:, :],
                            ets_[c // 2][:, (c % 2) * 512:(c % 2) * 512 + 512],
                            start=(t == 0), stop=(t == ST - 1),
                            skip_group_check=True)

                for t in range(ST):
                    pa = p_mm.tile([128, 1024], F32, name=f"ps_s{h}{t}a",
                                   tag="mm")
                    pb = p_mm.tile([128, 1024], F32, name=f"ps_s{h}{t}b",
                                   tag="mm")
                    lhsT = kT[off:off + 64, t * 128:(t + 1) * 128]
                    for c, (pt, o2) in enumerate(
                            ((pa, 0), (pa, 512), (pb, 0), (pb, 512))):
                        nc.tensor.matmul(
                            pt[:, o2:o2 + 512], lhsT,
                            qT[off:off + 64, c * 512:(c + 1) * 512],
                            start=True, stop=True)
                    if t == 0 and flush_prev[0] is not None:
                        flush_prev[0]()
                        flush_prev[0] = None
                    if t >= 2:
                        emit_ctx(t - 2)
                    zp = p_z.tile([128, 2], F32, name=f"zp{h}{t}", tag="zp",
                                  bufs=4)
                    et0 = p_et.tile([128, 1024], BF16, name=f"et{h}{t}a",
                                    tag="et", bufs=6)
                    et1 = p_et.tile([128, 1024], BF16, name=f"et{h}{t}b",
                                    tag="et", bufs=6)
                    nc.scalar.activation(et0[:], pa[:], EXP, scale=0.125,
                                         accum_out=zp[:, 0:1])
                    nc.scalar.activation(et1[:], pb[:], EXP, scale=0.125,
                                         accum_out=zp[:, 1:2])
                    u = fill.pop(0) if fill else None
                    if u is not None:
                        u()
                    z = p_z.tile([128, 1], F32, name=f"z{h}{t}", tag="z",
                                 bufs=4)
                    nc.vector.reduce_sum(z[:], zp[:], axis=mybir.AxisListType.X)
                    zr = p_z.tile([128, 1], F32, name=f"zr{h}{t}", tag="zr",
                                  bufs=4)
                    nc.vector.reciprocal(zr[:], z[:])
                    vs = p_z.tile([128, 64], BF16, name=f"vs{h}{t}",
                                  tag="vs", bufs=6)
                    nc.vector.tensor_scalar_mul(
                        vs[:], v_t[t][:, h * 64:(h + 1) * 64], zr[:])
                    hist.append(((et0, et1), vs))

                def mk_flush(cxa=cxa, pr=pr, off=off, emit_ctx=emit_ctx):
                    def fl():
                        emit_ctx(ST - 2)
                        emit_ctx(ST - 1)
                        # drain ctx into the paired fp16 accumulator
                        for half in range(2):
                            nc.vector.tensor_copy(
                                acc_t[pr][off:off + 64,
                                          half * 1024:(half + 1) * 1024],
                                cxa[half * 64:half * 64 + 64, :])
                    return fl
                flush_prev[0] = mk_flush()

            flush_prev[0]()

            # ---- pair-1 output projection tail ----
            for st in range(ST):
                out_unit(1, st, out1_d)

    nc.compile()
    return nc


_NC_CACHE = None


def _get_nc():
    global _NC_CACHE
    if _NC_CACHE is None:
        _NC_CACHE = build_nc()
    return _NC_CACHE


def _prep_in_maps(x, q_w, k_w, k_b, v_w, v_b, l_w):
    """Host-side sharding: per-core input dict (core = b*4 + g)."""
    f16 = np.float16
    in_maps = []
    xts = [np.ascontiguousarray(x[b].T.astype(f16)) for b in range(B)]
    ones = np.ones((1, 128), dtype=f16)
    for b in range(B):
        for g in range(4):
            hs = slice(g * HPC, (g + 1) * HPC)
            f0, f1 = g * HPC * P, (g + 1) * HPC * P
            in_maps.append({
                "xt": xts[b],
                "qwT": np.ascontiguousarray(
                    q_w[hs].transpose(2, 0, 1).reshape(D, HPC * P)
                    .astype(f16)),
                "kwT": np.ascontiguousarray(
                    k_w[hs].transpose(2, 0, 1).reshape(D, HPC * P)
                    .astype(f16)),
                "vwT": np.ascontiguousarray(
                    v_w[hs].transpose(2, 0, 1).reshape(D, HPC * P)
                    .astype(f16)),
                "kb": np.ascontiguousarray(k_b[hs].reshape(HPC * P, 1)),
                "vb": np.ascontiguousarray(v_b[hs].reshape(1, HPC * P)
                                           .astype(f16)),
                "lwT": np.ascontiguousarray(l_w[:, f0:f1].T.astype(f16)),
                "ones": ones,
            })
    return in_maps


def _run(inputs, trace=False):
    f32 = lambda a: np.asarray(a, dtype=np.float32)
    x = f32(inputs["x"])
    l_b = f32(inputs["l_b"])
    in_maps = _prep_in_maps(
        x, f32(inputs["q_w"]), f32(inputs["k_w"]), f32(inputs["k_b"]),
        f32(inputs["v_w"]), f32(inputs["v_b"]), f32(inputs["l_w"]))
    nc = _get_nc()
    res = run_bass_kernel_spmd(nc, in_maps, list(range(N_CORES)), trace=trace)
    out = np.empty((B, S, NUM_OUT), dtype=np.float32)
    for b in range(B):
        acc = None
        for g in range(4):
            r = res.results[b * 4 + g]
            part = r["out0"].astype(np.float32) + r["out1"].astype(np.float32)
            acc = part if acc is None else acc + part
        out[b] = acc + l_b
    return out, res


def kernel(**inputs):
    out, _ = _run(inputs, trace=False)
    return out
